# revision 25
# baseline (speedup 1.0000x reference)
"""Trainium2 Bass kernel for nn_MultiHeadAttention_81999515616076.

Reference computation (per batch b):
    xn = LN(x)                                    [N, IN]
    q  = xn @ W_q   -> [N, H, D]
    k,v= xn @ W_kv  -> [N, H, D] each
    ckv= LN(c_emb) @ W_ctx + b_ctx -> ck, cv      [M, D] (shared across heads)
    keys per head = [self keys (N)] + [null key] + [ctx keys (M)]  (2177 total)
    out = softmax(q.k / sqrt(D)) @ values         [N, H, D]
    y  = LN(out.reshape(N, H*D) @ W_out)          [N, IN]

Sharding (8 cores): core c -> batch b = c//4, head group g = c%4 (heads 4g..4g+3).
Per-core: LN+transpose of x, fp32r projections, flash-style attention for its 4
heads (scores computed transposed: [keys, tokens]; softmax denominator via a
ones-column in the PV matmul; no max subtraction -- scores are bounded ~N(0,0.4)).

Out-projection: instead of computing [2048,1024] partial sums and ReduceScatter
(8 MB of reduce-mode wire per core), each block's attnT activations are cast to
bf16 and exchanged with a single 8-rank AllToAll (64-token shards: rank j gets
tokens {512*blk + 64*j .. +64} of BOTH batches, 0.44 MB wire per block), then
every core runs the full 16-head out-projection + final LN on its own 128-row
slice (64 tokens x 2 batches per block).  Host only slices inputs / reassembles
outputs.
"""

import sys

sys.path.insert(0, "/opt/trn_rl_repo")

import numpy as np

import concourse.bacc as bacc
import concourse.tile as tile
import concourse.mybir as mybir
from concourse.masks import make_identity

B, N, IN = 2, 2048, 1024
H, D = 16, 64
CTX_DIM, M_CTX = 768, 128
NCORES = 8
HG = 4               # heads per core
FH = HG * D          # 256 local head-feats
BLK = 512            # token block
NBLK = N // BLK      # 4
KT = 17              # 16 self key tiles + 1 ctx key tile (null key handled separately)
SCALE = D ** -0.5    # 0.125
EPS = 1e-5

f32 = mybir.dt.float32
f32r = mybir.dt.float32r
bf16 = mybir.dt.bfloat16
AF = mybir.ActivationFunctionType
OP = mybir.AluOpType


def build_program():
    nc = bacc.Bacc("TRN2", target_bir_lowering=False, debug=False, num_devices=NCORES)

    # ---- per-core DRAM tensors (values sharded by host) ----
    x_d = nc.dram_tensor("x_loc", [N, IN], f32, kind="ExternalInput")
    wq_d = nc.dram_tensor("wq_loc", [IN, FH], bf16, kind="ExternalInput")
    wk_d = nc.dram_tensor("wk_loc", [IN, FH], bf16, kind="ExternalInput")
    wv_d = nc.dram_tensor("wv_loc", [IN, FH], bf16, kind="ExternalInput")
    wout_d = nc.dram_tensor("wout_loc", [H * D, IN], bf16, kind="ExternalInput")
    wctx_d = nc.dram_tensor("wctx", [CTX_DIM, 2 * D], bf16, kind="ExternalInput")
    cemb_d = nc.dram_tensor("cemb_loc", [M_CTX, CTX_DIM], f32, kind="ExternalInput")
    nullkv_d = nc.dram_tensor("nullkv", [2, D], bf16, kind="ExternalInput")
    lng_d = nc.dram_tensor("ln_g", [IN], f32, kind="ExternalInput")
    lnb_d = nc.dram_tensor("ln_b", [IN], bf16, kind="ExternalInput")
    ctxg_d = nc.dram_tensor("ctx_g", [CTX_DIM], f32, kind="ExternalInput")
    ctxb_d = nc.dram_tensor("ctx_b", [CTX_DIM], bf16, kind="ExternalInput")
    bctx_d = nc.dram_tensor("b_ctx", [2 * D], f32, kind="ExternalInput")
    outg_d = nc.dram_tensor("out_g", [IN], f32, kind="ExternalInput")
    outb_d = nc.dram_tensor("out_b", [IN], f32, kind="ExternalInput")
    ident_d = nc.dram_tensor("const_ident", [128, 128], bf16, kind="ExternalInput")
    ones_d = nc.dram_tensor("const_ones", [1, 128], bf16, kind="ExternalInput")
    y_out_d = nc.dram_tensor("y_out", [BLK, IN], f32, kind="ExternalOutput")
    # internal DRAM for the per-block AllToAll (separate tensors avoid WAR)
    a2a_in_d = [nc.dram_tensor(f"a2a_in{b}", [16 * 128, 64], bf16) for b in range(NBLK)]
    a2a_out_d = [nc.dram_tensor(f"a2a_out{b}", [16 * 128, 64], bf16) for b in range(NBLK)]

    with tile.TileContext(nc) as tc:
        _emit(nc, tc, locals())
    nc.compile()
    return nc


def _emit(nc, tc, t):
    from contextlib import ExitStack

    x_d, cemb_d = t["x_d"], t["cemb_d"]
    wq_d, wk_d, wv_d, wout_d, wctx_d = t["wq_d"], t["wk_d"], t["wv_d"], t["wout_d"], t["wctx_d"]
    nullkv_d, bctx_d = t["nullkv_d"], t["bctx_d"]
    lng_d, lnb_d, ctxg_d, ctxb_d = t["lng_d"], t["lnb_d"], t["ctxg_d"], t["ctxb_d"]
    outg_d, outb_d = t["outg_d"], t["outb_d"]
    y_out_d, a2a_in_d, a2a_out_d = t["y_out_d"], t["a2a_in_d"], t["a2a_out_d"]
    ident_d, ones_d = t["ident_d"], t["ones_d"]

    with ExitStack() as ctx:
        persist = ctx.enter_context(tc.tile_pool(name="persist", bufs=1))
        stat = ctx.enter_context(tc.tile_pool(name="stat", bufs=4))

        # ---------------- Phase 0: constants & weights ----------------
        ident = persist.tile([128, 128], bf16, name="ident", tag="ident")
        nc.sync.dma_start(ident, ident_d.ap())
        eps_t = persist.tile([128, 1], f32, name="eps", tag="eps")
        nc.vector.memset(eps_t, EPS)
        # int constants for the Newton-rsqrt (replaces Ln/Exp, whose
        # activation-table sets thrash against the softmax Exp set)
        i32 = mybir.dt.int32
        ones_i = persist.tile([128, 4], i32, name="ones_i", tag="ones_i")
        nc.vector.memset(ones_i, 1)
        magic_i = persist.tile([128, 4], i32, name="magic_i", tag="magic_i")
        nc.vector.memset(magic_i, 0x5F3759DF)

        def emit_rsqrt(out_ap, var_ap, c, pool):
            """out[128, c] = 1/sqrt(var + eps), on DVE only (quake seed +
            2 Newton steps; exact to ~1e-10 rel for the var ranges here)."""
            vb = pool.tile([128, c], f32, name="rs_vb", tag="rs_vb")
            nc.vector.tensor_scalar(vb, var_ap, eps_t[:, 0:1], None, op0=OP.add)
            y0 = pool.tile([128, c], f32, name="rs_y0", tag="rs_y0")
            nc.vector.tensor_tensor(y0.bitcast(i32), vb.bitcast(i32), ones_i[:, 0:c],
                                    op=OP.logical_shift_right)
            nc.vector.tensor_tensor(y0.bitcast(i32), magic_i[:, 0:c], y0.bitcast(i32),
                                    op=OP.subtract)
            t1 = pool.tile([128, c], f32, name="rs_t1", tag="rs_t1")
            y = y0
            for it in range(2):
                dst = out_ap if it == 1 else y0
                nc.vector.tensor_tensor(t1, y, y, op=OP.mult)
                nc.vector.scalar_tensor_tensor(t1, t1, -0.5, vb, op0=OP.mult, op1=OP.mult)
                nc.vector.scalar_tensor_tensor(dst, t1, 1.5, y, op0=OP.add, op1=OP.mult)

        # per-in-feature LN params as [128, chunks]
        g_sb = persist.tile([128, 8], f32, name="g_sb", tag="g_sb")
        nc.sync.dma_start(g_sb, lng_d.ap().rearrange("(c p) -> p c", p=128))
        lnb_sb = persist.tile([128, 8], bf16, name="lnb_sb", tag="lnb_sb")
        nc.sync.dma_start(lnb_sb, lnb_d.ap().rearrange("(c p) -> p c", p=128))
        ctxg_sb = persist.tile([128, 6], f32, name="ctxg_sb", tag="ctxg_sb")
        nc.sync.dma_start(ctxg_sb, ctxg_d.ap().rearrange("(c p) -> p c", p=128))
        ctxb_sb = persist.tile([128, 6], bf16, name="ctxb_sb", tag="ctxb_sb")
        nc.sync.dma_start(ctxb_sb, ctxb_d.ap().rearrange("(c p) -> p c", p=128))

        ones_ap = ones_d.ap()
        ones_r = persist.tile([1, 128], bf16, name="ones_r", tag="ones_r")
        nc.sync.dma_start(ones_r, ones_ap)
        ones2 = persist.tile([65, 64], bf16, name="ones2", tag="ones2")
        nc.sync.dma_start(ones2[64:65, :], ones_ap[0:1, 0:64])
        ones_hg = persist.tile([128, HG], bf16, name="ones_hg", tag="ones_hg")
        nc.sync.dma_start(ones_hg, ones_ap[0:1, 0:HG].to_broadcast([128, HG]))

        # null key/value: knull2 rows 0:64 and 64:128 both = null_k (for the two
        # row-packed head positions); nullv2 rows 0 and 32 = [null_v | 1].
        knull2 = persist.tile([128, 1], bf16, name="knull2", tag="knull2")
        nk_ap = nullkv_d.ap()[0:1, :].rearrange("a b -> b a")
        nc.sync.dma_start(knull2[0:64, :], nk_ap)
        nc.sync.dma_start(knull2[64:128, :], nk_ap)
        nullv2 = persist.tile([1, 65], bf16, name="nullv2", tag="nullv2")
        nv_ap = nullkv_d.ap()[1:2, :]
        nc.sync.dma_start(nullv2[0:1, 0:64], nv_ap)
        nc.sync.dma_start(nullv2[0:1, 64:65], ones_ap[0:1, 0:1])

        # Heavy P0 (weights + context projection), emitted AFTER block-0's LN/transpose
        # chains so the first x tiles hit the DMA queue first.
        wq_sb, wk_sb, wv_sb, wctx_sb, wout_sb = [], [], [], [], []
        cb_q, cb_k = [], []
        cv_row = persist.tile([1, FH], bf16, name="cv_row", tag="cv_row")
        ckvT_sb = persist.tile([128, M_CTX], bf16, name="ckvT", tag="ckvT")
        ck2 = persist.tile([128, M_CTX], bf16, name="ck2", tag="ck2")
        cv_ext = persist.tile([128, 65], bf16, name="cv_ext", tag="cv_ext")

        def emit_p0_heavy(p0sb, psP, psT):
            for name, dram, lst in (("wq", wq_d, wq_sb), ("wk", wk_d, wk_sb), ("wv", wv_d, wv_sb)):
                for c in range(8):
                    w = persist.tile([128, FH], bf16, name=f"{name}{c}", tag=f"{name}{c}")
                    nc.sync.dma_start(w, dram.ap()[128 * c : 128 * (c + 1), :])
                    nc.vector.tensor_scalar_mul(w, w, g_sb[:, c : c + 1])
                    lst.append(w)
            for c in range(6):
                w = persist.tile([128, 2 * D], bf16, name=f"wctx{c}", tag=f"wctx{c}")
                nc.sync.dma_start(w, wctx_d.ap()[128 * c : 128 * (c + 1), :])
                nc.vector.tensor_scalar_mul(w, w, ctxg_sb[:, c : c + 1])
                wctx_sb.append(w)
            for c in range(8):
                w = persist.tile([128, IN], bf16, name=f"wout{c}", tag=f"wout{c}")
                nc.sync.dma_start(w, wout_d.ap()[128 * c : 128 * (c + 1), :])
                wout_sb.append(w)
            # LN-beta folded biases: cb[j] = (ln_b @ W')[128j:128j+128] as [128,1]
            for wsb, lst in ((wq_sb, cb_q), (wk_sb, cb_k)):
                for j in range(2):
                    ps = psP.tile([128, 1], f32, name="p0bias", tag="proj")
                    for c in range(8):
                        nc.tensor.matmul(ps, wsb[c][:, 128 * j : 128 * (j + 1)],
                                         lnb_sb[:, c : c + 1], start=(c == 0), stop=(c == 7))
                    cb = persist.tile([128, 1], f32, name=f"cb{len(lst)}_{id(wsb) % 97}", tag=f"cb{len(cb_q)}_{len(cb_k)}")
                    nc.vector.tensor_copy(cb, ps)
                    lst.append(cb)
            # v bias as a row [1, FH] (added via a K=1 ones matmul)
            psc = psP.tile([1, FH], f32, name="p0cv", tag="proj")
            for c in range(8):
                nc.tensor.matmul(psc, lnb_sb[:, c : c + 1], wv_sb[c], start=(c == 0), stop=(c == 7))
            nc.vector.tensor_copy(cv_row, psc)
            # ---- context projection: ckv^T = W_ctx'.T @ LN(c_emb).T + bias ----
            cemb_sb = p0sb.tile([128, CTX_DIM], f32, name="cemb", tag="cemb")
            nc.sync.dma_start(cemb_sb, cemb_d.ap())
            stc = stat.tile([128, 3, 6], f32, name="stc", tag="stc")
            for i in range(3):
                nc.vector.bn_stats(stc[:, i, :], cemb_sb[:, 256 * i : 256 * (i + 1)])
            mvc = stat.tile([128, 2], f32, name="mvc", tag="mvc")
            nc.vector.bn_aggr(mvc, stc)
            rstd_c = stat.tile([128, 1], f32, name="rstd_c", tag="rstd_c")
            emit_rsqrt(rstd_c, mvc[:, 1:2], 1, stat)
            zc = p0sb.tile([128, CTX_DIM], bf16, name="zc", tag="zc")
            nc.vector.tensor_scalar(zc, cemb_sb, mvc[:, 0:1], rstd_c, op0=OP.subtract, op1=OP.mult)
            tpc = psT.tile([128, CTX_DIM], bf16, name="tpc", tag="tp")
            for c in range(6):
                nc.tensor.transpose(tpc[:, 128 * c : 128 * (c + 1)], zc[:, 128 * c : 128 * (c + 1)], ident)
            zcT = p0sb.tile([128, 6, 128], bf16, name="zcT", tag="zcT")
            nc.any.tensor_copy(zcT, tpc.rearrange("p (c w) -> p c w", c=6))
            # bias = (ctx_b @ W_ctx')^T + b_ctx
            psb2 = psP.tile([128, 1], f32, name="p0bias2", tag="proj")
            for c in range(6):
                nc.tensor.matmul(psb2, wctx_sb[c], ctxb_sb[:, c : c + 1],
                                 start=(c == 0), stop=(c == 5))
            bctx_sb = stat.tile([128, 1], f32, name="bctx_sb", tag="bctx_sb")
            nc.sync.dma_start(bctx_sb, bctx_d.ap().rearrange("(a p) -> p a", p=128))
            ckv_bias = stat.tile([128, 1], f32, name="ckv_bias", tag="ckv_bias")
            nc.vector.tensor_tensor(ckv_bias, psb2, bctx_sb, op=OP.add)
            psk = psP.tile([128, M_CTX], f32, name="psk", tag="proj")
            for c in range(6):
                nc.tensor.matmul(psk, wctx_sb[c], zcT[:, c, :], start=(c == 0), stop=(c == 5))
            nc.scalar.activation(ckvT_sb, psk, AF.Identity, bias=ckv_bias)
            # ck duplicated into both row-halves (for 2-head row packing)
            nc.sync.dma_start(ck2[0:64, :], ckvT_sb[0:64, :])
            nc.sync.dma_start(ck2[64:128, :], ckvT_sb[0:64, :])
            # cv in normal layout [M_CTX, 64] with a ones column -> [128, 65]
            cvT_tmp = p0sb.tile([64, M_CTX], bf16, name="cvT_tmp", tag="cvT_tmp")
            nc.sync.dma_start(cvT_tmp, ckvT_sb[64:128, :])
            ps_cv = psT.tile([128, 64], bf16, name="ps_cv", tag="tp")
            nc.tensor.transpose(ps_cv, cvT_tmp, ident[0:64, 0:64])
            nc.any.tensor_copy(cv_ext[:, 0:64], ps_cv)
            nc.vector.tensor_copy(cv_ext[:, 64:65], ones_hg[:, 0:1])

        # ---------------- persistent activation tensors ----------------
        qT = [persist.tile([128, N], bf16, name=f"qT{j}", tag=f"qT{j}") for j in range(2)]
        kT = [persist.tile([128, N], bf16, name=f"kT{j}", tag=f"kT{j}") for j in range(2)]
        attnT = [persist.tile([128, N], bf16, name=f"attnT{j}", tag=f"attnT{j}") for j in range(2)]
        v_tiles = []
        for i in range(16):
            vt = persist.tile([128, HG, 65], bf16, name=f"v{i}", tag=f"v{i}")
            nc.vector.tensor_copy(vt[:, :, 64:65], ones_hg.unsqueeze(2))
            v_tiles.append(vt)

        # ---------------- Phase 1: LN(x), transpose, q/k/v projections ----------------
        with tc.tile_pool(name="xp", bufs=3) as xp, \
             tc.tile_pool(name="zp", bufs=2) as zp, \
             tc.tile_pool(name="ztp", bufs=2) as ztp, \
             tc.tile_pool(name="p0sb", bufs=2) as p0sb, \
             tc.tile_pool(name="tpp", bufs=2, space="PSUM") as tpp, \
             tc.tile_pool(name="projp", bufs=2, space="PSUM") as projp, \
             tc.tile_pool(name="vpp", bufs=2, space="PSUM") as vpp:

            def emit_tts(blk):
                zT = ztp.tile([128, 8, BLK], bf16, name="zT", tag="zT")
                for tt in range(4):
                    t0 = BLK * blk + 128 * tt
                    x_t = xp.tile([128, IN], f32, name="x_t", tag="x_t")
                    nc.sync.dma_start(x_t, x_d.ap()[t0 : t0 + 128, :])
                    st = stat.tile([128, 2, 6], f32, name="st", tag="st")
                    nc.vector.bn_stats(st[:, 0, :], x_t[:, 0:512])
                    nc.vector.bn_stats(st[:, 1, :], x_t[:, 512:1024])
                    mv = stat.tile([128, 2], f32, name="mv", tag="mv")
                    nc.vector.bn_aggr(mv, st)
                    rstd = stat.tile([128, 1], f32, name="rstd", tag="rstd")
                    emit_rsqrt(rstd, mv[:, 1:2], 1, stat)
                    z_t = zp.tile([128, IN], bf16, name="z_t", tag="z_t")
                    nc.any.tensor_scalar(z_t, x_t, mv[:, 0:1], rstd, op0=OP.subtract, op1=OP.mult)
                    tp = tpp.tile([128, 1024], bf16, name="tp", tag="tp")
                    for c in range(8):
                        nc.tensor.transpose(tp[:, 128 * c : 128 * (c + 1)], z_t[:, 128 * c : 128 * (c + 1)], ident)
                    nc.any.tensor_copy(zT[:, :, 128 * tt : 128 * (tt + 1)], tp.rearrange("p (c w) -> p c w", c=8))
                return zT

            def emit_proj(blk, zT):
                # q/k projections (transposed layout), per head-pair j
                for wsb, cbs, dst in ((wq_sb, cb_q, qT), (wk_sb, cb_k, kT)):
                    for j in range(2):
                        ps = projp.tile([128, BLK], f32, name="proj", tag="proj")
                        for c in range(8):
                            nc.tensor.matmul(ps, wsb[c][:, 128 * j : 128 * (j + 1)], zT[:, c, :],
                                             start=(c == 0), stop=(c == 7))
                        nc.any.tensor_scalar_add(dst[j][:, BLK * blk : BLK * (blk + 1)], ps, cbs[j])
                # v projection (normal layout) per 128-token tile
                for tt in range(4):
                    psv = vpp.tile([128, FH], f32, name="psv", tag="psv")
                    for c in range(8):
                        nc.tensor.matmul(psv, zT[:, c, 128 * tt : 128 * (tt + 1)], wv_sb[c],
                                         start=(c == 0), stop=False)
                    nc.tensor.matmul(psv, ones_r, cv_row, start=False, stop=True)
                    vt = v_tiles[4 * blk + tt]
                    for hh in range(HG):
                        nc.any.tensor_copy(vt[:, hh, 0:64], psv[:, 64 * hh : 64 * (hh + 1)])

            zT0 = emit_tts(0)
            emit_p0_heavy(p0sb, projp, tpp)
            emit_proj(0, zT0)
            for blk in range(1, NBLK):
                zTb = emit_tts(blk)
                emit_proj(blk, zTb)

        # ---------------- Phases 2-4: attention, out-proj, chunked RS + final LN ----------------
        gout_rep = persist.tile([128, IN], f32, name="gout_rep", tag="gout_rep")
        nc.sync.dma_start(gout_rep, outg_d.ap().unsqueeze(0).to_broadcast([128, IN]))
        bout_rep = persist.tile([128, IN], f32, name="bout_rep", tag="bout_rep")
        nc.sync.dma_start(bout_rep, outb_d.ap().unsqueeze(0).to_broadcast([128, IN]))
        with tc.tile_pool(name="wtp", bufs=2) as wtp, \
             tc.tile_pool(name="oddp", bufs=2) as oddp, \
             tc.tile_pool(name="rcpp", bufs=2) as rcpp, \
             tc.tile_pool(name="expnp", bufs=2) as expnp, \
             tc.tile_pool(name="rtp", bufs=3) as rtp, \
             tc.tile_pool(name="fin", bufs=2) as fin, \
             tc.tile_pool(name="s0p", bufs=3, space="PSUM") as s0p, \
             tc.tile_pool(name="pvp", bufs=2, space="PSUM") as pvp:
            deferred = []
            deferred_fin = []
            for blk in range(NBLK):
                bsl = slice(BLK * blk, BLK * (blk + 1))
                for pj in range(2):
                    q0 = qT[pj][0:64, bsl]
                    q1 = qT[pj][64:128, bsl]
                    # null-key scores for both heads -> one psum row, one exp
                    expn = expnp.tile([1, 2 * BLK], bf16, name="expn", tag="expn")
                    ps_nl = s0p.tile([1, 2 * BLK], f32, name="ps_nl", tag="ps_s")
                    nc.tensor.matmul(ps_nl[0:1, 0:BLK], knull2[0:64, :], q0, start=True, stop=True)
                    nc.tensor.matmul(ps_nl[0:1, BLK : 2 * BLK], knull2[64:128, :], q1, start=True,
                                     stop=True, tile_position=(64, 0))
                    nc.scalar.activation(expn, ps_nl, AF.Exp, scale=SCALE)
                    # scores -> exp -> PV, pipelined per key tile; both heads share one
                    # [128,1024] scores psum + one exp op (h0 cols 0:512, h1 cols 512:1024).
                    # PV trails one key tile behind so PE never head-of-line blocks on exp.
                    ps_pv0 = pvp.tile([65, BLK], f32, name="ps_pv0", tag="ps_pv")
                    ps_pv1 = pvp.tile([65, BLK], f32, name="ps_pv1", tag="ps_pv")

                    def pv_step(kt, wt):
                        lv0 = cv_ext[:, 0:65] if kt == 16 else v_tiles[kt][:, 2 * pj, :]
                        lv1 = cv_ext[:, 0:65] if kt == 16 else v_tiles[kt][:, 2 * pj + 1, :]
                        nc.tensor.matmul(ps_pv0, lv0, wt[:, 0:BLK], start=(kt == 0), stop=False)
                        nc.tensor.matmul(ps_pv1, lv1, wt[:, BLK : 2 * BLK], start=(kt == 0), stop=False)

                    pending = []
                    for kt in range(KT):
                        if kt == 2 and deferred:
                            deferred.pop(0)()
                        if kt == 8 and pj == 1 and deferred_fin:
                            deferred_fin.pop(0)()
                        ps_s = s0p.tile([128, 2 * BLK], f32, name="ps_s", tag="ps_s")
                        wt = wtp.tile([128, 2 * BLK], bf16, name="wt", tag="wt", bufs=5)
                        l0 = ck2[0:64, :] if kt == 16 else kT[pj][0:64, 128 * kt : 128 * (kt + 1)]
                        l1 = ck2[64:128, :] if kt == 16 else kT[pj][64:128, 128 * kt : 128 * (kt + 1)]
                        nc.tensor.matmul(ps_s[:, 0:BLK], l0, q0, start=True, stop=True)
                        nc.tensor.matmul(ps_s[:, BLK : 2 * BLK], l1, q1, start=True, stop=True,
                                         tile_position=(64, 0))
                        if len(pending) >= 3:
                            pv_step(*pending.pop(0))
                        nc.scalar.activation(wt, ps_s, AF.Exp, scale=SCALE)
                        pending.append((kt, wt))
                    for args in pending:
                        pv_step(*args)
                    nc.tensor.matmul(ps_pv0, nullv2[0:1, :], expn[0:1, 0:BLK], start=False, stop=True)
                    nc.tensor.matmul(ps_pv1, nullv2[0:1, :], expn[0:1, BLK : 2 * BLK], start=False, stop=True)

                    # normalize: attnT = pv[0:64] * broadcast(1/denominator).  The recip
                    # (DVE) is emitted now so it overlaps the next pair's scores; the PE
                    # broadcast + multiply are deferred into the next pair's kt loop so
                    # the PE stream never head-of-line blocks on the DVE chain.
                    rcps = []
                    for h, ps_pv in ((0, ps_pv0), (1, ps_pv1)):
                        rcp = rcpp.tile([65, BLK], bf16, name="rcp", tag="rcp")
                        with nc.allow_low_precision(reason="bf16 recip of softmax denom; "
                                                    "uniform per-token scale cancels in final LN"):
                            nc.vector.reciprocal(rcp[64:65, :], ps_pv[64:65, :])
                        rcps.append(rcp)

                    def do_norm(pj=pj, bsl=bsl, pvs=(ps_pv0, ps_pv1), rcps=tuple(rcps)):
                        for h, (ps_pv, rcp) in enumerate(zip(pvs, rcps)):
                            ps_rb = s0p.tile([64, BLK], f32, name="ps_rb", tag="ps_s")
                            nc.tensor.matmul(ps_rb, ones2[64:65, :], rcp[64:65, :], start=True, stop=True,
                                             tile_position=(64, 0))
                            rb_sb = rcpp.tile([64, BLK], bf16, name="rb_sb", tag="rb_sb")
                            nc.vector.tensor_copy(rb_sb, ps_rb)
                            if h == 0:
                                nc.vector.tensor_tensor(attnT[pj][0:64, bsl], ps_pv[0:64, :], rb_sb, op=OP.mult)
                            else:
                                tmp = oddp.tile([64, BLK], bf16, name="odd", tag="odd")
                                nc.vector.tensor_tensor(tmp, ps_pv[0:64, :], rb_sb, op=OP.mult)
                                nc.sync.dma_start(attnT[pj][64:128, bsl], tmp)

                    deferred.append(do_norm)
                # flush pending normalizations, then stage this block's attnT
                # into the AllToAll input: shard j (rows 256j..256j+256) holds
                # tokens [512*blk + 64*j, +64) for heads (attnT0 | attnT1)
                while deferred:
                    deferred.pop(0)()
                for j in range(8):
                    csl = slice(BLK * blk + 64 * j, BLK * blk + 64 * (j + 1))
                    nc.sync.dma_start(a2a_in_d[blk].ap()[256 * j : 256 * j + 128, :],
                                      attnT[0][:, csl])
                    nc.sync.dma_start(a2a_in_d[blk].ap()[256 * j + 128 : 256 * (j + 1), :],
                                      attnT[1][:, csl])
                # 8-rank AllToAll: rank j receives [16 chunks x 128 feats, 64 tok]
                # (chunks 0-7 = batch of ranks 0-3, chunks 8-15 = batch of ranks 4-7)
                nc.gpsimd.collective_compute(
                    "AllToAll",
                    OP.bypass,
                    replica_groups=[[0, 1, 2, 3, 4, 5, 6, 7]],
                    ins=[a2a_in_d[blk].ap()],
                    outs=[a2a_out_d[blk].ap()],
                )
                # received activations -> SBUF [128, 8 fchunk, 2 batch, 64 tok];
                # gpsimd DMA queues behind the collective so its wait never
                # head-of-line blocks the compute engines
                rT = rtp.tile([128, 8, 2, 64], bf16, name="rT", tag="rT")
                for g in range(2):
                    nc.gpsimd.dma_start(
                        rT[:, :, g, :],
                        a2a_out_d[blk].ap()[1024 * g : 1024 * (g + 1), :]
                        .rearrange("(c p) w -> p c w", p=128))

                # full out-projection + final LN for this core's 128 rows of blk
                # (64 tokens of each batch) -- deferred a block so the A2A wait
                # never head-of-line blocks the compute engine queues
                def recv_block(blk=blk, rT=rT):
                    y_sb = fin.tile([128, IN], f32, name="y_sb", tag="y_sb")
                    for nh in range(2):
                        ps_y = s0p.tile([128, 512], f32, name="ps_y", tag="ps_s")
                        for c in range(8):
                            nc.tensor.matmul(ps_y, rT[:, c, :, :],
                                             wout_sb[c][:, 512 * nh : 512 * (nh + 1)],
                                             start=(c == 0), stop=(c == 7))
                        nc.vector.tensor_copy(y_sb[:, 512 * nh : 512 * (nh + 1)], ps_y)
                    st = stat.tile([128, 2, 6], f32, name="st", tag="st")
                    nc.vector.bn_stats(st[:, 0, :], y_sb[:, 0:512])
                    nc.vector.bn_stats(st[:, 1, :], y_sb[:, 512:1024])
                    mv = stat.tile([128, 2], f32, name="mv", tag="mv")
                    nc.vector.bn_aggr(mv, st)
                    rstd = stat.tile([128, 1], f32, name="rstd", tag="rstd")
                    emit_rsqrt(rstd, mv[:, 1:2], 1, stat)
                    zf = fin.tile([128, IN], f32, name="zf", tag="zf")
                    nc.vector.tensor_scalar(zf, y_sb, mv[:, 0:1], rstd, op0=OP.subtract, op1=OP.mult)
                    nc.vector.tensor_tensor(zf, zf, gout_rep, op=OP.mult)
                    nc.vector.tensor_tensor(zf, zf, bout_rep, op=OP.add)
                    nc.gpsimd.dma_start(y_out_d.ap()[128 * blk : 128 * (blk + 1), :], zf)

                deferred_fin.append(recv_block)
            while deferred_fin:
                deferred_fin.pop(0)()


def shard_inputs(inputs):
    """Split full inputs into 8 per-core input maps."""
    import ml_dtypes

    bf = ml_dtypes.bfloat16
    x = np.ascontiguousarray(np.asarray(inputs["x"], dtype=np.float32))
    c_emb = np.ascontiguousarray(np.asarray(inputs["c_emb"], dtype=np.float32))
    W_q = np.asarray(inputs["W_q"], np.float32).reshape(IN, H, D)
    W_kv = np.asarray(inputs["W_kv"], np.float32).reshape(IN, 2, H, D)
    W_out = np.ascontiguousarray(np.asarray(inputs["W_out"], np.float32).astype(bf))
    common = {
        "const_ident": np.eye(128, dtype=bf),
        "const_ones": np.ones((1, 128), bf),
        "wctx": np.ascontiguousarray(np.asarray(inputs["W_ctx"], np.float32).astype(bf)),
        "nullkv": np.ascontiguousarray(np.asarray(inputs["null_kv"], np.float32).astype(bf)),
        "ln_g": np.ascontiguousarray(np.asarray(inputs["ln_g"], np.float32)),
        "ln_b": np.ascontiguousarray(np.asarray(inputs["ln_b"], np.float32).astype(bf)),
        "ctx_g": np.ascontiguousarray(np.asarray(inputs["ctx_ln_g"], np.float32)),
        "ctx_b": np.ascontiguousarray(np.asarray(inputs["ctx_ln_b"], np.float32).astype(bf)),
        "b_ctx": np.ascontiguousarray(np.asarray(inputs["b_ctx"], np.float32)),
        "out_g": np.ascontiguousarray(np.asarray(inputs["out_ln_g"], np.float32)),
        "out_b": np.ascontiguousarray(np.asarray(inputs["out_ln_b"], np.float32)),
        "wout_loc": W_out,
    }
    in_maps = []
    for c in range(NCORES):
        b, g = c // 4, c % 4
        hs = slice(HG * g, HG * (g + 1))
        in_maps.append({
            "x_loc": x[b],
            "cemb_loc": c_emb[b],
            "wq_loc": np.ascontiguousarray(W_q[:, hs].reshape(IN, FH).astype(bf)),
            "wk_loc": np.ascontiguousarray(W_kv[:, 0, hs].reshape(IN, FH).astype(bf)),
            "wv_loc": np.ascontiguousarray(W_kv[:, 1, hs].reshape(IN, FH).astype(bf)),
            **common,
        })
    return in_maps


def unshard(results):
    out = np.empty((B, N, IN), np.float32)
    for j in range(NCORES):
        y = results[j]["y_out"]
        for blk in range(NBLK):
            t0 = BLK * blk + 64 * j
            out[0, t0 : t0 + 64, :] = y[128 * blk : 128 * blk + 64]
            out[1, t0 : t0 + 64, :] = y[128 * blk + 64 : 128 * (blk + 1)]
    return out


_CACHE = {}


def kernel(**inputs) -> np.ndarray:
    from concourse.bass_utils import run_bass_kernel_spmd

    if "nc" not in _CACHE:
        _CACHE["nc"] = build_program()
    nc = _CACHE["nc"]
    in_maps = shard_inputs(inputs)
    res = run_bass_kernel_spmd(nc, in_maps, list(range(NCORES))).results
    return unshard(res)


if __name__ == "__main__":
    nc = build_program()
    print("program built OK;",
          sum(1 for _ in nc.inst_map), "instructions")



# revision 27
# speedup vs baseline: 1.2069x; 1.2069x over previous
"""Trainium2 Bass kernel for nn_MultiHeadAttention_81999515616076.

Reference computation (per batch b):
    xn = LN(x)                                    [N, IN]
    q  = xn @ W_q   -> [N, H, D]
    k,v= xn @ W_kv  -> [N, H, D] each
    ckv= LN(c_emb) @ W_ctx + b_ctx -> ck, cv      [M, D] (shared across heads)
    keys per head = [self keys (N)] + [null key] + [ctx keys (M)]  (2177 total)
    out = softmax(q.k / sqrt(D)) @ values         [N, H, D]
    y  = LN(out.reshape(N, H*D) @ W_out)          [N, IN]

Sharding (8 cores): core c -> batch b = c//4, head group g = c%4 (heads 4g..4g+3).
Per-core: LN+transpose of x, fp32r projections, flash-style attention for its 4
heads (scores computed transposed: [keys, tokens]; softmax denominator via a
ones-column in the PV matmul; no max subtraction -- scores are bounded ~N(0,0.4)).

Out-projection: instead of computing [2048,1024] partial sums and ReduceScatter
(8 MB of reduce-mode wire per core), each block's attnT activations are cast to
bf16 and exchanged with a single 8-rank AllToAll (64-token shards: rank j gets
tokens {512*blk + 64*j .. +64} of BOTH batches, 0.44 MB wire per block), then
every core runs the full 16-head out-projection + final LN on its own 128-row
slice (64 tokens x 2 batches per block).  Host only slices inputs / reassembles
outputs.
"""

import sys

sys.path.insert(0, "/opt/trn_rl_repo")

import numpy as np

import concourse.bacc as bacc
import concourse.tile as tile
import concourse.mybir as mybir
from concourse.masks import make_identity

B, N, IN = 2, 2048, 1024
H, D = 16, 64
CTX_DIM, M_CTX = 768, 128
NCORES = 8
HG = 4               # heads per core
FH = HG * D          # 256 local head-feats
BLK = 512            # token block
NBLK = N // BLK      # 4
KT = 17              # 16 self key tiles + 1 ctx key tile (null key handled separately)
SCALE = D ** -0.5    # 0.125
EPS = 1e-5

f32 = mybir.dt.float32
f32r = mybir.dt.float32r
bf16 = mybir.dt.bfloat16
AF = mybir.ActivationFunctionType
OP = mybir.AluOpType


def build_program():
    nc = bacc.Bacc("TRN2", target_bir_lowering=False, debug=False, num_devices=NCORES)

    # ---- per-core DRAM tensors (values sharded by host) ----
    x_d = nc.dram_tensor("x_loc", [N, IN], f32, kind="ExternalInput")
    wq_d = nc.dram_tensor("wq_loc", [IN, FH], bf16, kind="ExternalInput")
    wk_d = nc.dram_tensor("wk_loc", [IN, FH], bf16, kind="ExternalInput")
    wv_d = nc.dram_tensor("wv_loc", [IN, FH], bf16, kind="ExternalInput")
    wout_d = nc.dram_tensor("wout_loc", [H * D, IN], bf16, kind="ExternalInput")
    wctx_d = nc.dram_tensor("wctx", [CTX_DIM, 2 * D], bf16, kind="ExternalInput")
    cemb_d = nc.dram_tensor("cemb_loc", [M_CTX, CTX_DIM], f32, kind="ExternalInput")
    nullkv_d = nc.dram_tensor("nullkv", [2, D], bf16, kind="ExternalInput")
    lng_d = nc.dram_tensor("ln_g", [IN], f32, kind="ExternalInput")
    lnb_d = nc.dram_tensor("ln_b", [IN], bf16, kind="ExternalInput")
    ctxg_d = nc.dram_tensor("ctx_g", [CTX_DIM], f32, kind="ExternalInput")
    ctxb_d = nc.dram_tensor("ctx_b", [CTX_DIM], bf16, kind="ExternalInput")
    bctx_d = nc.dram_tensor("b_ctx", [2 * D], f32, kind="ExternalInput")
    outg_d = nc.dram_tensor("out_g", [IN], f32, kind="ExternalInput")
    outb_d = nc.dram_tensor("out_b", [IN], f32, kind="ExternalInput")
    ident_d = nc.dram_tensor("const_ident", [128, 128], bf16, kind="ExternalInput")
    ones_d = nc.dram_tensor("const_ones", [1, 128], bf16, kind="ExternalInput")
    y_out_d = nc.dram_tensor("y_out", [BLK, IN], f32, kind="ExternalOutput")
    # internal DRAM for the per-block AllToAll (separate tensors avoid WAR)
    a2a_in_d = [nc.dram_tensor(f"a2a_in{b}", [16 * 128, 64], bf16) for b in range(NBLK)]
    a2a_out_d = [nc.dram_tensor(f"a2a_out{b}", [16 * 128, 64], bf16) for b in range(NBLK)]

    with tile.TileContext(nc) as tc:
        _emit(nc, tc, locals())
    nc.compile()
    return nc


def _emit(nc, tc, t):
    from contextlib import ExitStack

    x_d, cemb_d = t["x_d"], t["cemb_d"]
    wq_d, wk_d, wv_d, wout_d, wctx_d = t["wq_d"], t["wk_d"], t["wv_d"], t["wout_d"], t["wctx_d"]
    nullkv_d, bctx_d = t["nullkv_d"], t["bctx_d"]
    lng_d, lnb_d, ctxg_d, ctxb_d = t["lng_d"], t["lnb_d"], t["ctxg_d"], t["ctxb_d"]
    outg_d, outb_d = t["outg_d"], t["outb_d"]
    y_out_d, a2a_in_d, a2a_out_d = t["y_out_d"], t["a2a_in_d"], t["a2a_out_d"]
    ident_d, ones_d = t["ident_d"], t["ones_d"]

    with ExitStack() as ctx:
        persist = ctx.enter_context(tc.tile_pool(name="persist", bufs=1))
        stat = ctx.enter_context(tc.tile_pool(name="stat", bufs=4))

        # ---------------- Phase 0: constants & weights ----------------
        ident = persist.tile([128, 128], bf16, name="ident", tag="ident")
        nc.sync.dma_start(ident, ident_d.ap())
        eps_t = persist.tile([128, 1], f32, name="eps", tag="eps")
        nc.vector.memset(eps_t, EPS)
        # int constants for the Newton-rsqrt (replaces Ln/Exp, whose
        # activation-table sets thrash against the softmax Exp set)
        i32 = mybir.dt.int32
        ones_i = persist.tile([128, 4], i32, name="ones_i", tag="ones_i")
        nc.vector.memset(ones_i, 1)
        magic_i = persist.tile([128, 4], i32, name="magic_i", tag="magic_i")
        nc.vector.memset(magic_i, 0x5F3759DF)

        def emit_rsqrt(out_ap, var_ap, c, pool):
            """out[128, c] = 1/sqrt(var + eps), on DVE only (quake seed +
            2 Newton steps; exact to ~1e-10 rel for the var ranges here)."""
            vb = pool.tile([128, c], f32, name="rs_vb", tag="rs_vb")
            nc.vector.tensor_scalar(vb, var_ap, eps_t[:, 0:1], None, op0=OP.add)
            y0 = pool.tile([128, c], f32, name="rs_y0", tag="rs_y0")
            nc.vector.tensor_tensor(y0.bitcast(i32), vb.bitcast(i32), ones_i[:, 0:c],
                                    op=OP.logical_shift_right)
            nc.vector.tensor_tensor(y0.bitcast(i32), magic_i[:, 0:c], y0.bitcast(i32),
                                    op=OP.subtract)
            t1 = pool.tile([128, c], f32, name="rs_t1", tag="rs_t1")
            y = y0
            for it in range(2):
                dst = out_ap if it == 1 else y0
                nc.vector.tensor_tensor(t1, y, y, op=OP.mult)
                nc.vector.scalar_tensor_tensor(t1, t1, -0.5, vb, op0=OP.mult, op1=OP.mult)
                nc.vector.scalar_tensor_tensor(dst, t1, 1.5, y, op0=OP.add, op1=OP.mult)

        # per-in-feature LN params as [128, chunks]
        g_sb = persist.tile([128, 8], f32, name="g_sb", tag="g_sb")
        nc.sync.dma_start(g_sb, lng_d.ap().rearrange("(c p) -> p c", p=128))
        lnb_sb = persist.tile([128, 8], bf16, name="lnb_sb", tag="lnb_sb")
        nc.sync.dma_start(lnb_sb, lnb_d.ap().rearrange("(c p) -> p c", p=128))
        ctxg_sb = persist.tile([128, 6], f32, name="ctxg_sb", tag="ctxg_sb")
        nc.sync.dma_start(ctxg_sb, ctxg_d.ap().rearrange("(c p) -> p c", p=128))
        ctxb_sb = persist.tile([128, 6], bf16, name="ctxb_sb", tag="ctxb_sb")
        nc.sync.dma_start(ctxb_sb, ctxb_d.ap().rearrange("(c p) -> p c", p=128))

        ones_ap = ones_d.ap()
        ones_r = persist.tile([1, 128], bf16, name="ones_r", tag="ones_r")
        nc.sync.dma_start(ones_r, ones_ap)
        ones2 = persist.tile([65, 64], bf16, name="ones2", tag="ones2")
        nc.sync.dma_start(ones2[64:65, :], ones_ap[0:1, 0:64])
        ones_hg = persist.tile([128, HG], bf16, name="ones_hg", tag="ones_hg")
        nc.sync.dma_start(ones_hg, ones_ap[0:1, 0:HG].to_broadcast([128, HG]))

        # null key/value: knull2 rows 0:64 and 64:128 both = null_k (for the two
        # row-packed head positions); nullv2 rows 0 and 32 = [null_v | 1].
        knull2 = persist.tile([128, 1], bf16, name="knull2", tag="knull2")
        nk_ap = nullkv_d.ap()[0:1, :].rearrange("a b -> b a")
        nc.sync.dma_start(knull2[0:64, :], nk_ap)
        nc.sync.dma_start(knull2[64:128, :], nk_ap)
        nullv2 = persist.tile([1, 65], bf16, name="nullv2", tag="nullv2")
        nv_ap = nullkv_d.ap()[1:2, :]
        nc.sync.dma_start(nullv2[0:1, 0:64], nv_ap)
        nc.sync.dma_start(nullv2[0:1, 64:65], ones_ap[0:1, 0:1])

        # Heavy P0 (weights + context projection), emitted AFTER block-0's LN/transpose
        # chains so the first x tiles hit the DMA queue first.
        wq_sb, wk_sb, wv_sb, wctx_sb, wout_sb = [], [], [], [], []
        cb_q, cb_k = [], []
        cv_row = persist.tile([1, FH], bf16, name="cv_row", tag="cv_row")
        ckvT_sb = persist.tile([128, M_CTX], bf16, name="ckvT", tag="ckvT")
        ck2 = persist.tile([128, M_CTX], bf16, name="ck2", tag="ck2")
        cv_ext = persist.tile([128, 65], bf16, name="cv_ext", tag="cv_ext")

        def emit_p0_heavy(p0sb, psP, psT):
            for name, dram, lst in (("wq", wq_d, wq_sb), ("wk", wk_d, wk_sb), ("wv", wv_d, wv_sb)):
                for c in range(8):
                    w = persist.tile([128, FH], bf16, name=f"{name}{c}", tag=f"{name}{c}")
                    nc.sync.dma_start(w, dram.ap()[128 * c : 128 * (c + 1), :])
                    nc.vector.tensor_scalar_mul(w, w, g_sb[:, c : c + 1])
                    lst.append(w)
            for c in range(6):
                w = persist.tile([128, 2 * D], bf16, name=f"wctx{c}", tag=f"wctx{c}")
                nc.sync.dma_start(w, wctx_d.ap()[128 * c : 128 * (c + 1), :])
                nc.vector.tensor_scalar_mul(w, w, ctxg_sb[:, c : c + 1])
                wctx_sb.append(w)
            for c in range(8):
                w = persist.tile([128, IN], bf16, name=f"wout{c}", tag=f"wout{c}")
                nc.sync.dma_start(w, wout_d.ap()[128 * c : 128 * (c + 1), :])
                wout_sb.append(w)
            # LN-beta folded biases: cb[j] = (ln_b @ W')[128j:128j+128] as [128,1]
            for wsb, lst in ((wq_sb, cb_q), (wk_sb, cb_k)):
                for j in range(2):
                    ps = psP.tile([128, 1], f32, name="p0bias", tag="proj")
                    for c in range(8):
                        nc.tensor.matmul(ps, wsb[c][:, 128 * j : 128 * (j + 1)],
                                         lnb_sb[:, c : c + 1], start=(c == 0), stop=(c == 7))
                    cb = persist.tile([128, 1], f32, name=f"cb{len(lst)}_{id(wsb) % 97}", tag=f"cb{len(cb_q)}_{len(cb_k)}")
                    nc.vector.tensor_copy(cb, ps)
                    lst.append(cb)
            # v bias as a row [1, FH] (added via a K=1 ones matmul)
            psc = psP.tile([1, FH], f32, name="p0cv", tag="proj")
            for c in range(8):
                nc.tensor.matmul(psc, lnb_sb[:, c : c + 1], wv_sb[c], start=(c == 0), stop=(c == 7))
            nc.vector.tensor_copy(cv_row, psc)
            # ---- context projection: ckv^T = W_ctx'.T @ LN(c_emb).T + bias ----
            cemb_sb = p0sb.tile([128, CTX_DIM], f32, name="cemb", tag="cemb")
            nc.sync.dma_start(cemb_sb, cemb_d.ap())
            stc = stat.tile([128, 3, 6], f32, name="stc", tag="stc")
            for i in range(3):
                nc.vector.bn_stats(stc[:, i, :], cemb_sb[:, 256 * i : 256 * (i + 1)])
            mvc = stat.tile([128, 2], f32, name="mvc", tag="mvc")
            nc.vector.bn_aggr(mvc, stc)
            rstd_c = stat.tile([128, 1], f32, name="rstd_c", tag="rstd_c")
            emit_rsqrt(rstd_c, mvc[:, 1:2], 1, stat)
            zc = p0sb.tile([128, CTX_DIM], bf16, name="zc", tag="zc")
            nc.vector.tensor_scalar(zc, cemb_sb, mvc[:, 0:1], rstd_c, op0=OP.subtract, op1=OP.mult)
            tpc = psT.tile([128, CTX_DIM], bf16, name="tpc", tag="tp")
            for c in range(6):
                nc.tensor.transpose(tpc[:, 128 * c : 128 * (c + 1)], zc[:, 128 * c : 128 * (c + 1)], ident)
            zcT = p0sb.tile([128, 6, 128], bf16, name="zcT", tag="zcT")
            nc.any.tensor_copy(zcT, tpc.rearrange("p (c w) -> p c w", c=6))
            # bias = (ctx_b @ W_ctx')^T + b_ctx
            psb2 = psP.tile([128, 1], f32, name="p0bias2", tag="proj")
            for c in range(6):
                nc.tensor.matmul(psb2, wctx_sb[c], ctxb_sb[:, c : c + 1],
                                 start=(c == 0), stop=(c == 5))
            bctx_sb = stat.tile([128, 1], f32, name="bctx_sb", tag="bctx_sb")
            nc.sync.dma_start(bctx_sb, bctx_d.ap().rearrange("(a p) -> p a", p=128))
            ckv_bias = stat.tile([128, 1], f32, name="ckv_bias", tag="ckv_bias")
            nc.vector.tensor_tensor(ckv_bias, psb2, bctx_sb, op=OP.add)
            psk = psP.tile([128, M_CTX], f32, name="psk", tag="proj")
            for c in range(6):
                nc.tensor.matmul(psk, wctx_sb[c], zcT[:, c, :], start=(c == 0), stop=(c == 5))
            nc.scalar.activation(ckvT_sb, psk, AF.Identity, bias=ckv_bias)
            # ck duplicated into both row-halves (for 2-head row packing)
            nc.sync.dma_start(ck2[0:64, :], ckvT_sb[0:64, :])
            nc.sync.dma_start(ck2[64:128, :], ckvT_sb[0:64, :])
            # cv in normal layout [M_CTX, 64] with a ones column -> [128, 65]
            cvT_tmp = p0sb.tile([64, M_CTX], bf16, name="cvT_tmp", tag="cvT_tmp")
            nc.sync.dma_start(cvT_tmp, ckvT_sb[64:128, :])
            ps_cv = psT.tile([128, 64], bf16, name="ps_cv", tag="tp")
            nc.tensor.transpose(ps_cv, cvT_tmp, ident[0:64, 0:64])
            nc.any.tensor_copy(cv_ext[:, 0:64], ps_cv)
            nc.vector.tensor_copy(cv_ext[:, 64:65], ones_hg[:, 0:1])

        # ---------------- persistent activation tensors ----------------
        qT = [persist.tile([128, N], bf16, name=f"qT{j}", tag=f"qT{j}") for j in range(2)]
        kT = [persist.tile([128, N], bf16, name=f"kT{j}", tag=f"kT{j}") for j in range(2)]
        attnT = [persist.tile([128, N], bf16, name=f"attnT{j}", tag=f"attnT{j}") for j in range(2)]
        v_tiles = []
        for i in range(16):
            vt = persist.tile([128, HG, 65], bf16, name=f"v{i}", tag=f"v{i}")
            nc.vector.tensor_copy(vt[:, :, 64:65], ones_hg.unsqueeze(2))
            v_tiles.append(vt)

        # ---------------- Phase 1: LN(x), transpose, q/k/v projections ----------------
        with tc.tile_pool(name="xp", bufs=3) as xp, \
             tc.tile_pool(name="zp", bufs=2) as zp, \
             tc.tile_pool(name="ztp", bufs=2) as ztp, \
             tc.tile_pool(name="p0sb", bufs=2) as p0sb, \
             tc.tile_pool(name="tpp", bufs=2, space="PSUM") as tpp, \
             tc.tile_pool(name="projp", bufs=2, space="PSUM") as projp, \
             tc.tile_pool(name="vpp", bufs=2, space="PSUM") as vpp:

            def emit_tts(blk):
                zT = ztp.tile([128, 8, BLK], bf16, name="zT", tag="zT")
                for tt in range(4):
                    t0 = BLK * blk + 128 * tt
                    x_t = xp.tile([128, IN], f32, name="x_t", tag="x_t")
                    nc.sync.dma_start(x_t, x_d.ap()[t0 : t0 + 128, :])
                    st = stat.tile([128, 2, 6], f32, name="st", tag="st")
                    nc.vector.bn_stats(st[:, 0, :], x_t[:, 0:512])
                    nc.vector.bn_stats(st[:, 1, :], x_t[:, 512:1024])
                    mv = stat.tile([128, 2], f32, name="mv", tag="mv")
                    nc.vector.bn_aggr(mv, st)
                    rstd = stat.tile([128, 1], f32, name="rstd", tag="rstd")
                    emit_rsqrt(rstd, mv[:, 1:2], 1, stat)
                    z_t = zp.tile([128, IN], bf16, name="z_t", tag="z_t")
                    nc.any.tensor_scalar(z_t, x_t, mv[:, 0:1], rstd, op0=OP.subtract, op1=OP.mult)
                    tp = tpp.tile([128, 1024], bf16, name="tp", tag="tp")
                    for c in range(8):
                        nc.tensor.transpose(tp[:, 128 * c : 128 * (c + 1)], z_t[:, 128 * c : 128 * (c + 1)], ident)
                    nc.vector.tensor_copy(zT[:, :, 128 * tt : 128 * (tt + 1)], tp.rearrange("p (c w) -> p c w", c=8))
                return zT

            def emit_proj(blk, zT):
                # q/k projections (transposed layout), per head-pair j
                for wsb, cbs, dst in ((wq_sb, cb_q, qT), (wk_sb, cb_k, kT)):
                    for j in range(2):
                        ps = projp.tile([128, BLK], f32, name="proj", tag="proj")
                        for c in range(8):
                            nc.tensor.matmul(ps, wsb[c][:, 128 * j : 128 * (j + 1)], zT[:, c, :],
                                             start=(c == 0), stop=(c == 7))
                        nc.any.tensor_scalar_add(dst[j][:, BLK * blk : BLK * (blk + 1)], ps, cbs[j])
                # v projection (normal layout) per 128-token tile
                for tt in range(4):
                    psv = vpp.tile([128, FH], f32, name="psv", tag="psv")
                    for c in range(8):
                        nc.tensor.matmul(psv, zT[:, c, 128 * tt : 128 * (tt + 1)], wv_sb[c],
                                         start=(c == 0), stop=False)
                    nc.tensor.matmul(psv, ones_r, cv_row, start=False, stop=True)
                    vt = v_tiles[4 * blk + tt]
                    for hh in range(HG):
                        nc.vector.tensor_copy(vt[:, hh, 0:64], psv[:, 64 * hh : 64 * (hh + 1)])

            zT0 = emit_tts(0)
            emit_p0_heavy(p0sb, projp, tpp)
            emit_proj(0, zT0)
            for blk in range(1, NBLK):
                zTb = emit_tts(blk)
                emit_proj(blk, zTb)

        # ---------------- Phases 2-4: attention, out-proj, chunked RS + final LN ----------------
        gout_rep = persist.tile([128, IN], f32, name="gout_rep", tag="gout_rep")
        nc.sync.dma_start(gout_rep, outg_d.ap().unsqueeze(0).to_broadcast([128, IN]))
        bout_rep = persist.tile([128, IN], f32, name="bout_rep", tag="bout_rep")
        nc.sync.dma_start(bout_rep, outb_d.ap().unsqueeze(0).to_broadcast([128, IN]))
        with tc.tile_pool(name="wtp", bufs=2) as wtp, \
             tc.tile_pool(name="oddp", bufs=2) as oddp, \
             tc.tile_pool(name="rcpp", bufs=2) as rcpp, \
             tc.tile_pool(name="expnp", bufs=2) as expnp, \
             tc.tile_pool(name="rtp", bufs=3) as rtp, \
             tc.tile_pool(name="fin", bufs=2) as fin, \
             tc.tile_pool(name="s0p", bufs=3, space="PSUM") as s0p, \
             tc.tile_pool(name="pvp", bufs=2, space="PSUM") as pvp:
            deferred = []
            deferred_fin = []
            for blk in range(NBLK):
                bsl = slice(BLK * blk, BLK * (blk + 1))
                for pj in range(2):
                    q0 = qT[pj][0:64, bsl]
                    q1 = qT[pj][64:128, bsl]
                    # null-key scores for both heads -> one psum row, one exp
                    expn = expnp.tile([1, 2 * BLK], bf16, name="expn", tag="expn")
                    ps_nl = s0p.tile([1, 2 * BLK], f32, name="ps_nl", tag="ps_s")
                    nc.tensor.matmul(ps_nl[0:1, 0:BLK], knull2[0:64, :], q0, start=True, stop=True)
                    nc.tensor.matmul(ps_nl[0:1, BLK : 2 * BLK], knull2[64:128, :], q1, start=True,
                                     stop=True, tile_position=(64, 0))
                    nc.scalar.activation(expn, ps_nl, AF.Exp, scale=SCALE)
                    # scores -> exp -> PV, pipelined per key tile; both heads share one
                    # [128,1024] scores psum + one exp op (h0 cols 0:512, h1 cols 512:1024).
                    # PV trails one key tile behind so PE never head-of-line blocks on exp.
                    ps_pv0 = pvp.tile([65, BLK], f32, name="ps_pv0", tag="ps_pv")
                    ps_pv1 = pvp.tile([65, BLK], f32, name="ps_pv1", tag="ps_pv")

                    def pv_step(kt, wt):
                        lv0 = cv_ext[:, 0:65] if kt == 16 else v_tiles[kt][:, 2 * pj, :]
                        lv1 = cv_ext[:, 0:65] if kt == 16 else v_tiles[kt][:, 2 * pj + 1, :]
                        nc.tensor.matmul(ps_pv0, lv0, wt[:, 0:BLK], start=(kt == 0), stop=False)
                        nc.tensor.matmul(ps_pv1, lv1, wt[:, BLK : 2 * BLK], start=(kt == 0), stop=False)

                    pending = []
                    for kt in range(KT):
                        if kt == 2 and deferred:
                            deferred.pop(0)()
                        if kt == 8 and pj == 1 and deferred_fin:
                            deferred_fin.pop(0)()
                        ps_s = s0p.tile([128, 2 * BLK], f32, name="ps_s", tag="ps_s")
                        wt = wtp.tile([128, 2 * BLK], bf16, name="wt", tag="wt", bufs=5)
                        l0 = ck2[0:64, :] if kt == 16 else kT[pj][0:64, 128 * kt : 128 * (kt + 1)]
                        l1 = ck2[64:128, :] if kt == 16 else kT[pj][64:128, 128 * kt : 128 * (kt + 1)]
                        nc.tensor.matmul(ps_s[:, 0:BLK], l0, q0, start=True, stop=True)
                        nc.tensor.matmul(ps_s[:, BLK : 2 * BLK], l1, q1, start=True, stop=True,
                                         tile_position=(64, 0))
                        if len(pending) >= 3:
                            pv_step(*pending.pop(0))
                        nc.scalar.activation(wt, ps_s, AF.Exp, scale=SCALE)
                        pending.append((kt, wt))
                    for args in pending:
                        pv_step(*args)
                    nc.tensor.matmul(ps_pv0, nullv2[0:1, :], expn[0:1, 0:BLK], start=False, stop=True)
                    nc.tensor.matmul(ps_pv1, nullv2[0:1, :], expn[0:1, BLK : 2 * BLK], start=False, stop=True)

                    # normalize: attnT = pv[0:64] * broadcast(1/denominator).  The recip
                    # (DVE) is emitted now so it overlaps the next pair's scores; the PE
                    # broadcast + multiply are deferred into the next pair's kt loop so
                    # the PE stream never head-of-line blocks on the DVE chain.
                    rcps = []
                    for h, ps_pv in ((0, ps_pv0), (1, ps_pv1)):
                        rcp = rcpp.tile([65, BLK], bf16, name="rcp", tag="rcp")
                        with nc.allow_low_precision(reason="bf16 recip of softmax denom; "
                                                    "uniform per-token scale cancels in final LN"):
                            nc.vector.reciprocal(rcp[64:65, :], ps_pv[64:65, :])
                        rcps.append(rcp)

                    def do_norm(pj=pj, bsl=bsl, pvs=(ps_pv0, ps_pv1), rcps=tuple(rcps)):
                        for h, (ps_pv, rcp) in enumerate(zip(pvs, rcps)):
                            ps_rb = s0p.tile([64, BLK], f32, name="ps_rb", tag="ps_s")
                            nc.tensor.matmul(ps_rb, ones2[64:65, :], rcp[64:65, :], start=True, stop=True,
                                             tile_position=(64, 0))
                            rb_sb = rcpp.tile([64, BLK], bf16, name="rb_sb", tag="rb_sb")
                            nc.vector.tensor_copy(rb_sb, ps_rb)
                            if h == 0:
                                nc.vector.tensor_tensor(attnT[pj][0:64, bsl], ps_pv[0:64, :], rb_sb, op=OP.mult)
                            else:
                                tmp = oddp.tile([64, BLK], bf16, name="odd", tag="odd")
                                nc.vector.tensor_tensor(tmp, ps_pv[0:64, :], rb_sb, op=OP.mult)
                                nc.sync.dma_start(attnT[pj][64:128, bsl], tmp)

                    deferred.append(do_norm)
                # flush pending normalizations, then stage this block's attnT
                # into the AllToAll input: shard j (rows 256j..256j+256) holds
                # tokens [512*blk + 64*j, +64) for heads (attnT0 | attnT1)
                while deferred:
                    deferred.pop(0)()
                for j in range(8):
                    csl = slice(BLK * blk + 64 * j, BLK * blk + 64 * (j + 1))
                    nc.sync.dma_start(a2a_in_d[blk].ap()[256 * j : 256 * j + 128, :],
                                      attnT[0][:, csl])
                    nc.sync.dma_start(a2a_in_d[blk].ap()[256 * j + 128 : 256 * (j + 1), :],
                                      attnT[1][:, csl])
                # 8-rank AllToAll: rank j receives [16 chunks x 128 feats, 64 tok]
                # (chunks 0-7 = batch of ranks 0-3, chunks 8-15 = batch of ranks 4-7)
                nc.gpsimd.collective_compute(
                    "AllToAll",
                    OP.bypass,
                    replica_groups=[[0, 1, 2, 3, 4, 5, 6, 7]],
                    ins=[a2a_in_d[blk].ap()],
                    outs=[a2a_out_d[blk].ap()],
                )
                # received activations -> SBUF [128, 8 fchunk, 2 batch, 64 tok];
                # gpsimd DMA queues behind the collective so its wait never
                # head-of-line blocks the compute engines
                rT = rtp.tile([128, 8, 2, 64], bf16, name="rT", tag="rT")
                for g in range(2):
                    nc.gpsimd.dma_start(
                        rT[:, :, g, :],
                        a2a_out_d[blk].ap()[1024 * g : 1024 * (g + 1), :]
                        .rearrange("(c p) w -> p c w", p=128))

                # full out-projection + final LN for this core's 128 rows of blk
                # (64 tokens of each batch) -- deferred a block so the A2A wait
                # never head-of-line blocks the compute engine queues
                def recv_block(blk=blk, rT=rT):
                    y_sb = fin.tile([128, IN], f32, name="y_sb", tag="y_sb")
                    for nh in range(2):
                        ps_y = s0p.tile([128, 512], f32, name="ps_y", tag="ps_s")
                        for c in range(8):
                            nc.tensor.matmul(ps_y, rT[:, c, :, :],
                                             wout_sb[c][:, 512 * nh : 512 * (nh + 1)],
                                             start=(c == 0), stop=(c == 7))
                        nc.vector.tensor_copy(y_sb[:, 512 * nh : 512 * (nh + 1)], ps_y)
                    st = stat.tile([128, 2, 6], f32, name="st", tag="st")
                    nc.vector.bn_stats(st[:, 0, :], y_sb[:, 0:512])
                    nc.vector.bn_stats(st[:, 1, :], y_sb[:, 512:1024])
                    mv = stat.tile([128, 2], f32, name="mv", tag="mv")
                    nc.vector.bn_aggr(mv, st)
                    rstd = stat.tile([128, 1], f32, name="rstd", tag="rstd")
                    emit_rsqrt(rstd, mv[:, 1:2], 1, stat)
                    zf = fin.tile([128, IN], f32, name="zf", tag="zf")
                    nc.vector.tensor_scalar(zf, y_sb, mv[:, 0:1], rstd, op0=OP.subtract, op1=OP.mult)
                    nc.vector.tensor_tensor(zf, zf, gout_rep, op=OP.mult)
                    nc.vector.tensor_tensor(zf, zf, bout_rep, op=OP.add)
                    nc.gpsimd.dma_start(y_out_d.ap()[128 * blk : 128 * (blk + 1), :], zf)

                deferred_fin.append(recv_block)
            while deferred_fin:
                deferred_fin.pop(0)()


def shard_inputs(inputs):
    """Split full inputs into 8 per-core input maps."""
    import ml_dtypes

    bf = ml_dtypes.bfloat16
    x = np.ascontiguousarray(np.asarray(inputs["x"], dtype=np.float32))
    c_emb = np.ascontiguousarray(np.asarray(inputs["c_emb"], dtype=np.float32))
    W_q = np.asarray(inputs["W_q"], np.float32).reshape(IN, H, D)
    W_kv = np.asarray(inputs["W_kv"], np.float32).reshape(IN, 2, H, D)
    W_out = np.ascontiguousarray(np.asarray(inputs["W_out"], np.float32).astype(bf))
    common = {
        "const_ident": np.eye(128, dtype=bf),
        "const_ones": np.ones((1, 128), bf),
        "wctx": np.ascontiguousarray(np.asarray(inputs["W_ctx"], np.float32).astype(bf)),
        "nullkv": np.ascontiguousarray(np.asarray(inputs["null_kv"], np.float32).astype(bf)),
        "ln_g": np.ascontiguousarray(np.asarray(inputs["ln_g"], np.float32)),
        "ln_b": np.ascontiguousarray(np.asarray(inputs["ln_b"], np.float32).astype(bf)),
        "ctx_g": np.ascontiguousarray(np.asarray(inputs["ctx_ln_g"], np.float32)),
        "ctx_b": np.ascontiguousarray(np.asarray(inputs["ctx_ln_b"], np.float32).astype(bf)),
        "b_ctx": np.ascontiguousarray(np.asarray(inputs["b_ctx"], np.float32)),
        "out_g": np.ascontiguousarray(np.asarray(inputs["out_ln_g"], np.float32)),
        "out_b": np.ascontiguousarray(np.asarray(inputs["out_ln_b"], np.float32)),
        "wout_loc": W_out,
    }
    in_maps = []
    for c in range(NCORES):
        b, g = c // 4, c % 4
        hs = slice(HG * g, HG * (g + 1))
        in_maps.append({
            "x_loc": x[b],
            "cemb_loc": c_emb[b],
            "wq_loc": np.ascontiguousarray(W_q[:, hs].reshape(IN, FH).astype(bf)),
            "wk_loc": np.ascontiguousarray(W_kv[:, 0, hs].reshape(IN, FH).astype(bf)),
            "wv_loc": np.ascontiguousarray(W_kv[:, 1, hs].reshape(IN, FH).astype(bf)),
            **common,
        })
    return in_maps


def unshard(results):
    out = np.empty((B, N, IN), np.float32)
    for j in range(NCORES):
        y = results[j]["y_out"]
        for blk in range(NBLK):
            t0 = BLK * blk + 64 * j
            out[0, t0 : t0 + 64, :] = y[128 * blk : 128 * blk + 64]
            out[1, t0 : t0 + 64, :] = y[128 * blk + 64 : 128 * (blk + 1)]
    return out


_CACHE = {}


def kernel(**inputs) -> np.ndarray:
    from concourse.bass_utils import run_bass_kernel_spmd

    if "nc" not in _CACHE:
        _CACHE["nc"] = build_program()
    nc = _CACHE["nc"]
    in_maps = shard_inputs(inputs)
    res = run_bass_kernel_spmd(nc, in_maps, list(range(NCORES))).results
    return unshard(res)


if __name__ == "__main__":
    nc = build_program()
    print("program built OK;",
          sum(1 for _ in nc.inst_map), "instructions")

